# revision 13
# baseline (speedup 1.0000x reference)
"""Trainium2 Bass kernel for batched multi-head attention that also
materializes the attention-probability matrix.

Problem shapes (hardcoded): q,k,v [4,16,2048,64] f32, mask [4,1,1,2048] f32
(1.0 = masked). Returns (out [4,16,2048,64], attn [4,16,2048,2048]).

Sharding: the 64 (batch, head) pairs are split 8-per-core across 8
NeuronCores (core c gets batch c//2, heads (c%2)*8 .. +8). Each core runs an
identical program on its own slice; no cross-core communication.

Per (b,h) pair, on-core algorithm:
  prep:   DMA Q,K,V; PE-transpose Q,K into [64,2048] "d-major" layout,
          augmented with a 65th contraction row (Q-side: -1e9 constant,
          K-side: raw mask) so the additive mask folds into the matmul.
  A:      scores_T[k,q] = Kaug.T @ Qaug (fp32r), exp on ACT into SBUF,
          then out_T[d,q] += V_kt.T @ exp_T accumulated in PSUM per 2-kt
          group and drained to an SBUF accumulator by DVE.
  C:      scores[q,k] = Qaug.T @ Kaug, exp on ACT with accum_out giving the
          softmax row-sums for free, DVE reciprocal + in-place normalize,
          2 MiB DMA writes of attn.
  B:      PE-transpose out_T chunks, scale rows by 1/rowsum, DMA out.
"""

import numpy as np
from contextlib import ExitStack

import concourse.bass as bass
import concourse.tile as tile
from concourse import bacc, mybir
from concourse.bass_utils import run_bass_kernel_spmd

F32 = mybir.dt.float32
F32R = mybir.dt.float32r
I32 = mybir.dt.int32
EXP = mybir.ActivationFunctionType.Exp

B, H, L, D = 4, 16, 2048, 64
NCORES = 8
PAIRS = (B * H) // NCORES        # 8 (b,h) pairs per core
CORES_PER_B = NCORES // B        # 2
HPC = H // CORES_PER_B           # 8 heads per core
NEG = -1.0e9


def build_program(pairs=PAIRS, seq=L, n_cores=NCORES):
    """Build + compile the per-core Bass program. Parameterized so tests can
    build a small variant for the simulator."""
    nt = seq // 128                       # number of 128-row tiles
    nj = seq // 512                       # number of 512-wide matmul slices
    nh = seq // 1024                      # number of 1024-wide psum halves

    nc = bacc.Bacc(
        "TRN2",
        target_bir_lowering=False,
        debug=False,
        enable_asserts=False,
        num_devices=n_cores,
    )
    q_d = nc.dram_tensor("q", [pairs, seq, D], F32, kind="ExternalInput").ap()
    k_d = nc.dram_tensor("k", [pairs, seq, D], F32, kind="ExternalInput").ap()
    v_d = nc.dram_tensor("v", [pairs, seq, D], F32, kind="ExternalInput").ap()
    m_d = nc.dram_tensor("mask", [1, seq], F32, kind="ExternalInput").ap()
    attn_d = nc.dram_tensor("attn", [pairs, seq, seq], F32, kind="ExternalOutput").ap()
    out_d = nc.dram_tensor("out", [pairs, seq, D], F32, kind="ExternalOutput").ap()

    with tile.TileContext(nc) as tc:
        with ExitStack() as ctx:
            _emit(ctx, tc, q_d, k_d, v_d, m_d, attn_d, out_d, pairs, seq, nt, nj, nh)

    nc.compile()
    return nc


def _emit(ctx, tc, q_d, k_d, v_d, m_d, attn_d, out_d, pairs, seq, nt, nj, nh):
    nc = tc.nc
    scale = 1.0 / np.sqrt(D)

    psum = ctx.enter_context(tc.tile_pool(name="psum", bufs=4, space="PSUM"))
    consts = ctx.enter_context(tc.tile_pool(name="consts", bufs=1))
    pairp = ctx.enter_context(tc.tile_pool(name="pairp", bufs=2))
    expp = ctx.enter_context(tc.tile_pool(name="expp", bufs=4))
    stagep = ctx.enter_context(tc.tile_pool(name="stagep", bufs=3))
    smallp = ctx.enter_context(tc.tile_pool(name="smallp", bufs=4))

    # 128x128 fp32 identity for PE transposes: col_idx == partition_idx.
    iota_j = consts.tile([128, 128], F32)
    nc.gpsimd.iota(
        iota_j, pattern=[[1, 128]], base=0, channel_multiplier=0,
        allow_small_or_imprecise_dtypes=True,
    )
    iota_p = consts.tile([128, 1], F32)
    nc.gpsimd.iota(
        iota_p, pattern=[[0, 1]], base=0, channel_multiplier=1,
        allow_small_or_imprecise_dtypes=True,
    )
    ident = consts.tile([128, 128], F32)
    nc.vector.tensor_scalar(
        ident, iota_j, iota_p[:, 0:1], None, op0=mybir.AluOpType.is_equal
    )
    mask_row = consts.tile([1, seq], F32)
    nc.sync.dma_start(mask_row, m_d)
    neg_row = consts.tile([1, seq], F32)
    nc.gpsimd.memset(neg_row, NEG)

    def ps_tile(name):
        return psum.tile([128, 1024], F32, name=name, tag="ps")

    def prep(p):
        """Load pair p's inputs and build Qaug/Kaug [65, seq] transposed."""
        qs = pairp.tile([128, nt, D], F32, name="qs", tag="qs")
        nc.sync.dma_start(qs, q_d[p].rearrange("(t p) d -> p t d", p=128))
        ks = pairp.tile([128, nt, D], F32, name="ks", tag="ks")
        nc.sync.dma_start(ks, k_d[p].rearrange("(t p) d -> p t d", p=128))
        vs = pairp.tile([128, nt, D], F32, name="vs", tag="vs")
        nc.sync.dma_start(vs, v_d[p].rearrange("(t p) d -> p t d", p=128))

        qt_aug = pairp.tile([65, seq], F32R, name="qt_aug", tag="qt_aug")
        kt_aug = pairp.tile([65, seq], F32R, name="kt_aug", tag="kt_aug")
        # 65th contraction row: (-1e9) * mask[k] added to every score.
        nc.vector.tensor_copy(qt_aug[64:65, :], neg_row)
        nc.vector.tensor_copy(kt_aug[64:65, :], mask_row)
        vsr = pairp.tile([128, nt, D], F32R, name="vsr", tag="vsr")
        nc.vector.tensor_copy(vsr, vs)

        for src, dst in ((qs, qt_aug), (ks, kt_aug)):
            for h in range(nh):
                tp = ps_tile("tp_prep")
                for t in range(8):
                    nc.tensor.transpose(
                        tp[0:64, t * 128:(t + 1) * 128],
                        src[:, 8 * h + t, :],
                        ident,
                    )
                nc.vector.tensor_copy(
                    dst[0:64, h * 1024:(h + 1) * 1024], tp[0:64, :]
                )
        return qt_aug, kt_aug, vsr

    def phase_a(qt_aug, kt_aug, vs, acc):
        """scores_T -> exp_T -> out_T accumulation (acc [64, seq] SBUF)."""
        qr = qt_aug
        kr = kt_aug
        for g in range(nt // 2):
            es = []
            for kt in (2 * g, 2 * g + 1):
                e = expp.tile([128, seq], F32R, name="e", tag="e")
                for h in range(nh):
                    sc = ps_tile("sc_a")
                    for j in range(2):
                        nc.tensor.matmul(
                            sc[:, j * 512:(j + 1) * 512],
                            kr[:, kt * 128:(kt + 1) * 128],
                            qr[:, (2 * h + j) * 512:(2 * h + j + 1) * 512],
                            start=True,
                            stop=True,
                        )
                    nc.scalar.activation(
                        e[:, h * 1024:(h + 1) * 1024], sc, EXP, scale=scale
                    )
                es.append((kt, e))
            for h in range(nh):
                av = ps_tile("av")
                for i, (kt, e) in enumerate(es):
                    for j in range(2):
                        nc.tensor.matmul(
                            av[0:64, j * 512:(j + 1) * 512],
                            vs[:, kt, :],
                            e[:, (2 * h + j) * 512:(2 * h + j + 1) * 512],
                            start=(i == 0),
                            stop=(i == len(es) - 1),
                        )
                if g == 0:
                    nc.vector.tensor_copy(
                        acc[:, h * 1024:(h + 1) * 1024], av[0:64, :]
                    )
                else:
                    nc.vector.tensor_add(
                        acc[:, h * 1024:(h + 1) * 1024],
                        acc[:, h * 1024:(h + 1) * 1024],
                        av[0:64, :],
                    )

    def phase_c(p, qt_aug, kt_aug, recip):
        """scores -> exp(+rowsum) -> normalize -> attn DMA."""
        qr = qt_aug
        kr = kt_aug
        attn_v = attn_d[p].rearrange("(t p) k -> p t k", p=128)
        for qq in range(nt // 2):
            st = stagep.tile([128, 2, seq], F32, name="st", tag="st")
            for i in range(2):
                qt = 2 * qq + i
                rs = smallp.tile([128, max(nh, 2)], F32, name="rs", tag="rs")
                for h in range(nh):
                    sc = ps_tile("sc_c")
                    for j in range(2):
                        nc.tensor.matmul(
                            sc[:, j * 512:(j + 1) * 512],
                            qr[:, qt * 128:(qt + 1) * 128],
                            kr[:, (2 * h + j) * 512:(2 * h + j + 1) * 512],
                            start=True,
                            stop=True,
                        )
                    nc.scalar.activation(
                        st[:, i, h * 1024:(h + 1) * 1024],
                        sc,
                        EXP,
                        scale=scale,
                        accum_out=rs[:, h:h + 1],
                    )
                if nh == 2:
                    rsum = smallp.tile([128, 1], F32, name="rsum", tag="rsum")
                    nc.vector.tensor_add(rsum, rs[:, 0:1], rs[:, 1:2])
                else:
                    rsum = rs[:, 0:1]
                nc.vector.reciprocal(recip[:, qt:qt + 1], rsum)
                nc.vector.tensor_scalar_mul(
                    st[:, i, :], st[:, i, :], recip[:, qt:qt + 1]
                )
            nc.sync.dma_start(attn_v[:, 2 * qq:2 * qq + 2, :], st)

    def phase_b(p, acc, recip):
        """Transpose out_T, scale by 1/rowsum, DMA out."""
        outsb = pairp.tile([128, nt, D], F32, name="outsb", tag="outsb")
        ch = ps_tile("ch_b")
        for qt in range(nt):
            nc.tensor.transpose(
                ch[:, qt * 64:(qt + 1) * 64],
                acc[:, qt * 128:(qt + 1) * 128],
                ident[0:64, 0:64],
            )
            nc.vector.tensor_scalar_mul(
                outsb[:, qt, :], ch[:, qt * 64:(qt + 1) * 64], recip[:, qt:qt + 1]
            )
        nc.sync.dma_start(out_d[p].rearrange("(t p) d -> p t d", p=128), outsb)

    handles = prep(0)
    for p in range(pairs):
        qt_aug, kt_aug, vs = handles
        if p + 1 < pairs:
            handles = prep(p + 1)
        acc = pairp.tile([64, seq], F32, name="acc", tag="acc")
        phase_a(qt_aug, kt_aug, vs, acc)
        recip = pairp.tile([128, nt], F32, name="recip", tag="recip")
        phase_c(p, qt_aug, kt_aug, recip)
        phase_b(p, acc, recip)


_cached_nc = None


def _get_nc():
    global _cached_nc
    if _cached_nc is None:
        _cached_nc = build_program()
    return _cached_nc


last_run = None


def build_in_maps(inputs):
    q = np.ascontiguousarray(inputs["q"], dtype=np.float32)
    k = np.ascontiguousarray(inputs["k"], dtype=np.float32)
    v = np.ascontiguousarray(inputs["v"], dtype=np.float32)
    mask = np.ascontiguousarray(inputs["mask"], dtype=np.float32)
    in_maps = []
    for c in range(NCORES):
        b = c // CORES_PER_B
        h0 = (c % CORES_PER_B) * HPC
        in_maps.append(
            {
                "q": q[b, h0:h0 + HPC].reshape(PAIRS, L, D),
                "k": k[b, h0:h0 + HPC].reshape(PAIRS, L, D),
                "v": v[b, h0:h0 + HPC].reshape(PAIRS, L, D),
                "mask": mask[b].reshape(1, L),
            }
        )
    return in_maps


def kernel(q, k, v, mask, trace=False):
    """Full-input entry point: shards across 8 cores, runs, gathers."""
    global last_run
    nc = _get_nc()
    in_maps = build_in_maps({"q": q, "k": k, "v": v, "mask": mask})

    res = run_bass_kernel_spmd(
        nc, in_maps, core_ids=list(range(NCORES)), trace=trace
    )
    last_run = res

    out = np.empty((B, H, L, D), dtype=np.float32)
    attn = np.empty((B, H, L, L), dtype=np.float32)
    for c in range(NCORES):
        b = c // CORES_PER_B
        h0 = (c % CORES_PER_B) * HPC
        out[b, h0:h0 + HPC] = res.results[c]["out"].reshape(HPC, L, D)
        attn[b, h0:h0 + HPC] = res.results[c]["attn"].reshape(HPC, L, L)
    return out, attn
